# revision 16
# baseline (speedup 1.0000x reference)
"""Trainium2 Bass kernel for nn_DecoderCrossAttention.

Reference computation (per voxel v, batch b):
    q = Wq x_v + bq                        (x = decoder_features, [C])
    k_j = Wk y_jv + bk, v_j = Wv y_jv + bv (y = skip features, COND=4 frames)
    s_j[h] = <q_h, k_jh> / sqrt(DH)        (NH=8 heads of DH=16)
    attn = softmax_j(s)                    (over the 4 conditioning frames)
    o = Wo (sum_j attn_j * v_j) + bo + x_v
    out = GroupNorm8(o) * gamma + beta     (stats over (C/G, H, W, D) per batch)

Strategy (8 NeuronCores, data-parallel over H; each core: 2*4*32*32 voxels):
  * Feature-major layout [C=128 partitions, voxels free], 512-voxel tiles.
  * All projections are PE matmuls (f32r full-rate at N=512; Wo in bf16).
  * Per-head score reduction and the softmax head->channel broadcast are PE
    matmuls against 0/1 masks built in-kernel with iota+compare.
  * Scores of a 4-tile chunk are packed into ONE [128,512] PSUM bank using
    32-partition strips (tile_position), so exp / Z / 1/Z run once per chunk
    at full 128-partition width:
        esb = exp(S/4); Z = strip-mask matmul (replicated per cond block);
        1/Z = exp(-ln Z) on the scalar engine (vector reciprocal is ~8
        cycles/elem; the two table activations are ~4x cheaper).
  * attn*V products feed 4 accumulating bf16 output-projection matmuls.
  * Residual + bias + per-channel GN sums fused in one scalar_tensor_tensor;
    sum-of-squares via scalar-engine Square with accumulate.
  * GroupNorm is global: per-channel sum/sumsq AllReduce (1KB) per batch,
    batch 0's collective+rescale hidden under batch 1 compute. The rescale
    is in-place on the accumulator, alternating scalar/vector engines.

The walrus build here accepts only ONE sync wait per instruction; Tile
attaches many.  split_waits() hoists extras onto standalone EventSemaphore
instructions post-scheduling.
"""

import sys

if "/opt/trn_rl_repo" not in sys.path:
    sys.path.insert(0, "/opt/trn_rl_repo")

import numpy as np

B, COND, C, H, W, D = 2, 4, 128, 32, 32, 32
NH, DH, G = 8, 16, 8
EPS = 1e-5
NCORES = 8
HS = H // NCORES          # 4 H-planes per core
NVOX = HS * W * D         # 4096 voxels per batch per core
NT = 512                  # voxels per tile
NTILES = NVOX // NT       # 8 tiles per batch
NK = B * NTILES           # 16 tiles per rep
N_GROUP = (C // G) * H * W * D   # elements per (batch, group) for GN stats

_CACHE = {}


def _split_waits(nc):
    """Hoist extra sync waits onto standalone EventSemaphore instructions."""
    from concourse import mybir
    import bass_rust

    n_split = 0
    for func in nc.m.functions:
        for blk in func.blocks:
            new_list = []
            changed = False
            for inst in blk.instructions:
                si = inst.sync_info
                waits = list(si.on_wait) if si is not None else []
                if len(waits) > 1:
                    changed = True
                    for w in waits[:-1]:
                        ev = mybir.InstEventSemaphore(
                            name=f"wsplit-{nc.next_id()}", ins=[], outs=[]
                        )
                        ev.engine = inst.engine
                        ev.sync_info = bass_rust.SyncInfo(on_wait=[w], on_update=[])
                        new_list.append(ev)
                        n_split += 1
                    inst.sync_info = bass_rust.SyncInfo(
                        on_wait=[waits[-1]], on_update=list(si.on_update)
                    )
                new_list.append(inst)
            if changed:
                blk.instructions = new_list
    return n_split


def _build(n_reps=1):
    import concourse.bass as bass
    import concourse.tile as tile
    from concourse import mybir
    from contextlib import ExitStack

    dt = mybir.dt
    f32 = dt.float32
    f32r = dt.float32r
    bf16 = dt.bfloat16
    i32 = dt.int32
    Alu = mybir.AluOpType
    Act = mybir.ActivationFunctionType
    ts = bass.ts

    nc = bass.Bass("TRN2", target_bir_lowering=False, debug=False,
                   num_devices=NCORES)
    x_io = nc.dram_tensor("x", [B, C, NVOX], f32r, kind="ExternalInput").ap()
    y_io = nc.dram_tensor("y", [B, COND, C, NVOX], f32r, kind="ExternalInput").ap()
    w_io = {}
    for name in ("wq", "wk", "wv", "wo"):
        w_io[name] = nc.dram_tensor(name, [C, C], f32r, kind="ExternalInput").ap()
    v_io = {}
    for name in ("bq", "bk", "bv", "bo", "gamma", "beta"):
        v_io[name] = nc.dram_tensor(name, [C, 1], f32, kind="ExternalInput").ap()
    out_io = nc.dram_tensor("out", [B, C, NVOX], f32, kind="ExternalOutput").ap()

    def mm(out, lhsT, rhs, start=True, stop=True):
        nc.tensor.matmul(out, lhsT=lhsT, rhs=rhs, start=start, stop=stop)

    with tile.TileContext(nc) as tc, ExitStack() as ctx:
        # ---------------- constants / weights / masks -------------------
        const = ctx.enter_context(tc.tile_pool(name="const", bufs=1))
        dram = ctx.enter_context(tc.tile_pool(name="dram", bufs=1, space="DRAM"))

        vecs = {}
        for name, io in v_io.items():
            t = const.tile([C, 1], f32, tag=f"vec_{name}")
            nc.sync.dma_start(t[:], io[:])
            vecs[name] = t

        with tc.tile_pool(name="setup", bufs=1) as setup:
            def icast(dst_ap, src_ap):
                nc.vector.tensor_copy(dst_ap, src_ap)

            # partition-index and free-index helpers
            p128 = setup.tile([C, C], i32, tag="p128")
            nc.gpsimd.iota(p128[:], pattern=[[0, C]], base=0, channel_multiplier=1)
            f128 = setup.tile([C, C], i32, tag="f128")
            nc.gpsimd.iota(f128[:], pattern=[[1, C]], base=0, channel_multiplier=0)
            hc128 = setup.tile([C, C], i32, tag="hc128")
            nc.vector.tensor_scalar(hc128[:], p128[:], 4, None,
                                    Alu.arith_shift_right)
            tmpi = setup.tile([C, C], i32, tag="tmpi")

            # identity [128,128] (for PE transpose of the weights)
            ident = const.tile([C, C], f32r, tag="ident")
            nc.vector.tensor_tensor(tmpi[:], f128[:], p128[:], Alu.is_equal)
            icast(ident[:], tmpi[:])

            # All matmul outputs must start at partition 0, so the per-chunk
            # score / broadcast masks are full-height [128,128] slices, one
            # per (tile-strip u, cond j), packed along free as [128, 16*128].
            NM = 4 * COND * C
            pfull = setup.tile([C, NM], i32, tag="pfull")
            nc.gpsimd.iota(pfull[:], pattern=[[0, NM]], base=0,
                           channel_multiplier=1)
            p16f = setup.tile([C, NM], i32, tag="p16f")
            nc.vector.tensor_scalar(p16f[:], pfull[:], 4, None,
                                    Alu.arith_shift_right)
            # base(u,j) = 32u + 8j replicated over the 128-col block
            ujb = setup.tile([C, NM], i32, tag="ujb")
            nc.gpsimd.iota(ujb[:].rearrange("p (u j c) -> p u j c", u=4, j=4),
                           pattern=[[32, 4], [8, 4], [0, C]], base=0,
                           channel_multiplier=0)
            # r = col index within each 128 block
            rcol = setup.tile([C, NM], i32, tag="rcol")
            nc.gpsimd.iota(rcol[:].rearrange("p (b c) -> p b c", b=16),
                           pattern=[[0, 16], [1, C]], base=0,
                           channel_multiplier=0)
            # mask32f[p, (u,j,r)] = 1 iff r == 32u + 8j + p//16  (score lhsT)
            tgt = setup.tile([C, NM], i32, tag="tgt")
            nc.vector.tensor_tensor(tgt[:], ujb[:], p16f[:], Alu.add)
            ebig = setup.tile([C, NM], i32, tag="ebig")
            nc.vector.tensor_tensor(ebig[:], rcol[:], tgt[:], Alu.is_equal)
            mask32f = const.tile([C, NM], bf16, tag="mask32f")
            icast(mask32f[:], ebig[:])

            # maskbf[p, (u,j,c)] = 1 iff p == 32u + 8j + c//16  (bcast lhsT)
            r16 = setup.tile([C, NM], i32, tag="r16")
            nc.vector.tensor_scalar(r16[:], rcol[:], 4, None,
                                    Alu.arith_shift_right)
            nc.vector.tensor_tensor(tgt[:], ujb[:], r16[:], Alu.add)
            nc.vector.tensor_tensor(ebig[:], pfull[:], tgt[:], Alu.is_equal)
            maskbf = const.tile([C, NM], bf16, tag="maskbf")
            icast(maskbf[:], ebig[:])

            # zquad [128,128]: 1 iff p//32==r//32 and p%8==r%8 (chunk Z matmul)
            rr = setup.tile([C, C], i32, tag="rr")
            nc.gpsimd.iota(rr[:], pattern=[[1, C]], base=0, channel_multiplier=0)
            za = setup.tile([C, C], i32, tag="za")
            nc.vector.tensor_scalar(za[:], rr[:], 5, None, Alu.arith_shift_right)
            zp = setup.tile([C, C], i32, tag="zp")
            nc.vector.tensor_scalar(zp[:], p128[:], 5, None, Alu.arith_shift_right)
            zeq1 = setup.tile([C, C], i32, tag="zeq1")
            nc.vector.tensor_tensor(zeq1[:], za[:], zp[:], Alu.is_equal)
            zm1 = setup.tile([C, C], i32, tag="zm1")
            nc.vector.tensor_scalar(zm1[:], rr[:], 3, 3,
                                    Alu.arith_shift_right, Alu.arith_shift_left)
            nc.vector.tensor_tensor(zm1[:], rr[:], zm1[:], Alu.subtract)
            zm2 = setup.tile([C, C], i32, tag="zm2")
            nc.vector.tensor_scalar(zm2[:], p128[:], 3, 3,
                                    Alu.arith_shift_right, Alu.arith_shift_left)
            nc.vector.tensor_tensor(zm2[:], p128[:], zm2[:], Alu.subtract)
            zeq2 = setup.tile([C, C], i32, tag="zeq2")
            nc.vector.tensor_tensor(zeq2[:], zm1[:], zm2[:], Alu.is_equal)
            nc.vector.tensor_tensor(zeq1[:], zeq1[:], zeq2[:], Alu.mult)
            zquad = const.tile([C, C], f32r, tag="zquad")
            icast(zquad[:], zeq1[:])

            # gmask [128, 8]: 1 iff c//16 == g   (GN group reduction)
            g8 = setup.tile([C, 8], i32, tag="g8")
            nc.gpsimd.iota(g8[:], pattern=[[1, 8]], base=0, channel_multiplier=0)
            e8 = setup.tile([C, 8], i32, tag="e8")
            nc.vector.tensor_tensor(e8[:], g8[:], hc128[:, 0:8], Alu.is_equal)
            gmask = const.tile([C, 8], f32, tag="gmask")
            icast(gmask[:], e8[:])

            # gm2 [8, 128]: 1 iff p == c//16    (GN group -> channel bcast)
            p8 = setup.tile([8, C], i32, tag="p8")
            nc.gpsimd.iota(p8[:], pattern=[[0, C]], base=0, channel_multiplier=1)
            fc8 = setup.tile([8, C], i32, tag="fc8")
            nc.gpsimd.iota(fc8[:], pattern=[[1, C]], base=0, channel_multiplier=0)
            nc.vector.tensor_scalar(fc8[:], fc8[:], 4, None, Alu.arith_shift_right)
            e82 = setup.tile([8, C], i32, tag="e82")
            nc.vector.tensor_tensor(e82[:], p8[:], fc8[:], Alu.is_equal)
            gm2 = const.tile([8, C], f32, tag="gm2")
            icast(gm2[:], e82[:])

            # --- load + transpose the four projection weights
            wT = {}
            with tc.tile_pool(name="psum_setup", bufs=1, space="PSUM") as psum_su:
                raws = {}
                for name in ("wq", "wk", "wv", "wo"):
                    raw = setup.tile([C, C], f32r, tag=f"raw_{name}")
                    nc.sync.dma_start(raw[:], w_io[name][:])
                    raws[name] = raw
                for name in ("wq", "wk", "wv", "wo"):
                    pst = psum_su.tile([C, C], f32r, tag=f"pst_{name}")
                    nc.tensor.transpose(pst[:], raws[name][:], ident[:])
                    t = const.tile([C, C], bf16 if name == "wo" else f32r,
                                   tag=f"wT_{name}")
                    nc.scalar.copy(t[:], pst[:])
                    wT[name] = t

            # warm the activation tables used by the steady state
            warm = setup.tile([C, 2], f32, tag="warm")
            nc.vector.memset(warm[:, 0:1], 1.0)
            nc.scalar.activation(warm[:, 1:2], warm[:, 0:1], Act.Exp)
            nc.scalar.activation(warm[:, 1:2], warm[:, 0:1], Act.Ln)
            nc.scalar.activation(warm[:, 1:2], warm[:, 0:1], Act.Square)
            nc.scalar.activation(warm[:, 1:2], warm[:, 0:1], Act.Sqrt)
            nc.scalar.activation(warm[:, 1:2], warm[:, 0:1], Act.Identity)

        # ---------------- main pipeline ---------------------------------
        p = dict(
            xres=ctx.enter_context(tc.tile_pool(name="xres", bufs=2)),
            ypool=ctx.enter_context(tc.tile_pool(name="ypool", bufs=2)),
            qpool=ctx.enter_context(tc.tile_pool(name="qpool", bufs=2)),
            qkpool=ctx.enter_context(tc.tile_pool(name="qkpool", bufs=2)),
            vpool=ctx.enter_context(tc.tile_pool(name="vpool", bufs=5)),
            wpool=ctx.enter_context(tc.tile_pool(name="wpool", bufs=2)),
            soft=ctx.enter_context(tc.tile_pool(name="soft", bufs=2)),
            opool=ctx.enter_context(tc.tile_pool(name="opool", bufs=1)),
            stats=ctx.enter_context(tc.tile_pool(name="stats", bufs=1)),
            ps_k=ctx.enter_context(tc.tile_pool(name="ps_k", bufs=2, space="PSUM")),
            ps_q=ctx.enter_context(tc.tile_pool(name="ps_q", bufs=1, space="PSUM")),
            ps_v=ctx.enter_context(tc.tile_pool(name="ps_v", bufs=1, space="PSUM")),
            ps_sz=ctx.enter_context(tc.tile_pool(name="ps_sz", bufs=1, space="PSUM")),
            ps_bb=ctx.enter_context(tc.tile_pool(name="ps_bb", bufs=2, space="PSUM")),
            ps_o=ctx.enter_context(tc.tile_pool(name="ps_o", bufs=1, space="PSUM")),
        )

        # chunk layout: 4 tiles per chunk; chunk-local index u selects the
        # 32-partition strip of the packed score / Z tiles
        def chunk_of(t):
            return t // 4, t % 4

        NCH = NTILES // 4          # chunks per batch

        for rep in range(n_reps):
            out_acc = p["opool"].tile([C, B * NVOX], f32, tag="out_acc")
            sums = p["stats"].tile([C, B * NTILES], f32, tag="sums")
            ssqs = p["stats"].tile([C, B * NTILES], f32, tag="ssqs")
            dump = p["stats"].tile([C, NT], f32, tag="dump")

            tiles = [(b, t) for b in range(B) for t in range(NTILES)]
            xres_b = {}
            ych_state = {}
            fstate = {}
            chunk_ps = {}
            sstate = {}
            cc_state = {}

            def front(k):
                b, t = tiles[k]
                if t == 0:
                    xr = p["xres"].tile([C, NVOX], f32r, tag="xres")
                    nc.sync.dma_start(xr[:, 0: NVOX // 2],
                                      x_io[b][:, 0: NVOX // 2])
                    nc.sync.dma_start(xr[:, NVOX // 2: NVOX],
                                      x_io[b][:, NVOX // 2: NVOX])
                    xres_b[b] = xr
                if t % 2 == 0:
                    ych = p["ypool"].tile([C, COND * 2 * NT], f32r, tag="ych")
                    ysrc = y_io[b].rearrange("j c v -> c j v")
                    nc.sync.dma_start(
                        ych[:].rearrange("p (j v) -> p j v", j=COND),
                        ysrc[:, :, ts(t // 2, 2 * NT)])
                    ych_state[b] = ych
                ych = ych_state[b]
                yj = lambda j: ych[:, j * 2 * NT + (t % 2) * NT:
                               j * 2 * NT + (t % 2 + 1) * NT]
                xt = xres_b[b][:, ts(t, NT)]
                ci, u = chunk_of(t)
                cid = b * NCH + ci

                psQ = p["ps_q"].tile([C, NT], f32, tag="q")
                mm(psQ[:], wT["wq"][:], xt)
                qsb = p["qpool"].tile([C, NT], f32, tag="qsb")
                nc.scalar.activation(qsb[:], psQ[:], Act.Identity,
                                     bias=vecs["bq"][:])

                qk = p["qkpool"].tile([C, COND * NT], bf16, tag="qk")
                for j in range(COND):
                    psK = p["ps_k"].tile([C, NT], f32, tag="k")
                    mm(psK[:], wT["wk"][:], yj(j))
                    nc.vector.scalar_tensor_tensor(
                        qk[:, ts(j, NT)], psK[:], vecs["bk"][:], qsb[:],
                        Alu.add, Alu.mult)

                if u == 0:
                    psS4c = p["ps_sz"].tile([C, NT], f32, tag="sz", name="psS4")
                    chunk_ps[cid] = psS4c
                psS4 = chunk_ps[cid]
                for j in range(COND):
                    nc.tensor.matmul(
                        psS4[:], lhsT=mask32f[:, ts(u * COND + j, C)],
                        rhs=qk[:, ts(j, NT)],
                        start=(u == 0 and j == 0),
                        stop=(u == 3 and j == COND - 1),
                        skip_group_check=True)

                vb = p["vpool"].tile([C, COND * NT], bf16, tag="vb")
                for j in range(COND):
                    psV = p["ps_v"].tile([C, NT], f32, tag="v")
                    mm(psV[:], wT["wv"][:], yj(j))
                    nc.scalar.activation(vb[:, ts(j, NT)], psV[:],
                                         Act.Identity, bias=vecs["bv"][:])
                fstate[k] = (xt, vb)

            def softchunk(cid):
                psS4 = chunk_ps.pop(cid)
                esb = p["soft"].tile([C, NT], f32r, tag="esb")
                nc.scalar.activation(esb[:], psS4[:], Act.Exp, scale=0.25)
                psZ = p["ps_sz"].tile([C, NT], f32, tag="sz")
                mm(psZ[:], zquad[:], esb[:])
                lnz = p["soft"].tile([C, NT], f32, tag="lnz")
                nc.scalar.activation(lnz[:], psZ[:], Act.Ln)
                rsb = p["soft"].tile([C, NT], f32, tag="rsb")
                nc.scalar.activation(rsb[:], lnz[:], Act.Exp, scale=-1.0)
                etsb = p["soft"].tile([C, NT], bf16, tag="etsb")
                nc.vector.tensor_tensor(etsb[:], esb[:].bitcast(f32), rsb[:],
                                        Alu.mult)
                sstate[cid] = etsb

            def back(k):
                b, t = tiles[k]
                ci, u = chunk_of(t)
                xt, vb = fstate.pop(k)
                etsb = sstate[b * NCH + ci]
                wb = p["wpool"].tile([C, COND * NT], bf16, tag="wb")
                for j in range(COND):
                    psBB = p["ps_bb"].tile([C, NT], f32, tag="bb")
                    mm(psBB[:], maskbf[:, ts(u * COND + j, C)], etsb[:])
                    nc.vector.tensor_tensor(wb[:, ts(j, NT)], psBB[:],
                                            vb[:, ts(j, NT)], Alu.mult)
                psO = p["ps_o"].tile([C, NT], f32, tag="o")
                for j in range(COND):
                    mm(psO[:], wT["wo"][:], wb[:, ts(j, NT)],
                       start=(j == 0), stop=(j == COND - 1))
                col = b * NTILES + t
                outt = out_acc[:, col * NT: (col + 1) * NT]
                nc.vector.scalar_tensor_tensor(
                    outt, psO[:], vecs["bo"][:], xt.bitcast(f32),
                    Alu.add, Alu.add,
                    accum_out=sums[:, col: col + 1])
                nc.scalar.activation(
                    dump[:], outt, Act.Square,
                    accum_out=ssqs[:, col: col + 1])

            def gn_pre(b):
                """Reduce per-channel stats and launch the AllReduce."""
                ccsb = p["stats"].tile([C, 2], f32, tag=f"ccsb{b}")
                nc.vector.reduce_sum(ccsb[:, 0:1],
                                     sums[:, b * NTILES:(b + 1) * NTILES],
                                     axis=mybir.AxisListType.X)
                nc.vector.reduce_sum(ccsb[:, 1:2],
                                     ssqs[:, b * NTILES:(b + 1) * NTILES],
                                     axis=mybir.AxisListType.X)
                cc_in = dram.tile([C, 2], f32, tag=f"cc_in{b}")
                cc_out = dram.tile([C, 2], f32, tag=f"cc_out{b}")
                nc.sync.dma_start(cc_in[:], ccsb[:])
                nc.gpsimd.collective_compute(
                    "AllReduce", Alu.add,
                    replica_groups=[list(range(NCORES))],
                    ins=[cc_in.opt()], outs=[cc_out.opt()])
                cc_state[b] = cc_out

            def gn_post(b):
                """Stats -> per-channel affine -> rescale out_acc -> store."""
                cc_out = cc_state.pop(b)
                gsb = p["stats"].tile([C, 2], f32, tag=f"gsb{b}")
                nc.sync.dma_start(gsb[:], cc_out[:])
                psG = p["ps_q"].tile([8, 2], f32, tag="q")
                nc.tensor.matmul(psG[:], lhsT=gmask[:], rhs=gsb[:],
                                 start=True, stop=True)
                msb = p["stats"].tile([8, 2], f32, tag=f"msb{b}")
                nc.vector.tensor_scalar(msb[:], psG[:], 1.0 / N_GROUP, None,
                                        Alu.mult)
                vtmp = p["stats"].tile([8, 2], f32, tag=f"vtmp{b}")
                eps_t = p["stats"].tile([8, 1], f32, tag=f"eps{b}")
                nc.vector.memset(eps_t[:], EPS)
                nc.vector.tensor_tensor(vtmp[:, 0:1], msb[:, 0:1],
                                        msb[:, 0:1], Alu.mult)
                nc.vector.tensor_tensor(vtmp[:, 1:2], msb[:, 1:2],
                                        vtmp[:, 0:1], Alu.subtract)
                nc.scalar.activation(vtmp[:, 0:1], vtmp[:, 1:2], Act.Sqrt,
                                     bias=eps_t[:])
                pstat = p["stats"].tile([8, 2], f32, tag=f"pstat{b}")
                nc.vector.tensor_copy(pstat[:, 0:1], msb[:, 0:1])
                nc.vector.reciprocal(pstat[:, 1:2], vtmp[:, 0:1])
                psP = p["ps_q"].tile([C, 2], f32, tag="q")
                nc.tensor.matmul(psP[:], lhsT=gm2[:], rhs=pstat[:],
                                 start=True, stop=True)
                scale_b = p["stats"].tile([C, 1], f32, tag=f"scale{b}")
                nc.vector.tensor_tensor(scale_b[:], psP[:, 1:2],
                                        vecs["gamma"][:], Alu.mult)
                negb_b = p["stats"].tile([C, 1], f32, tag=f"negb{b}")
                nc.vector.scalar_tensor_tensor(
                    negb_b[:], psP[:, 0:1], scale_b[:],
                    vecs["beta"][:], Alu.mult, Alu.subtract)
                nb2 = p["stats"].tile([C, 1], f32, tag=f"nb2{b}")
                nc.vector.tensor_scalar(nb2[:], negb_b[:], -1.0, None,
                                        Alu.mult)
                for t in range(NTILES):
                    seg = out_acc[:, (b * NTILES + t) * NT:
                                  (b * NTILES + t + 1) * NT]
                    if t % 2 == 0:
                        nc.scalar.activation(seg, seg, Act.Identity,
                                             bias=nb2[:], scale=scale_b[:])
                    else:
                        nc.vector.tensor_scalar(
                            seg, seg, scale_b[:], negb_b[:],
                            Alu.mult, Alu.subtract)
                    if t % 4 == 3:
                        half = 4 * NT
                        hi = (t // 4)
                        nc.sync.dma_start(
                            out_io[b][:, hi * half: (hi + 1) * half],
                            out_acc[:, b * NVOX + hi * half:
                                    b * NVOX + (hi + 1) * half])

            for k in range(NK + 4):
                if k < NK:
                    front(k)
                if k >= 4:
                    back(k - 4)
                    bdone, tdone = tiles[k - 4]
                    if tdone == NTILES - 1:
                        gn_pre(bdone)
                if k < NK:
                    bk_, tk_ = tiles[k]
                    ci_, u_ = chunk_of(tk_)
                    if u_ == 3:
                        softchunk(bk_ * NCH + ci_)
                # hide batch-0 AllReduce under batch-1 compute
                if k == NK - 1 and 0 in cc_state:
                    gn_post(0)
            gn_post(B - 1)

    _split_waits(nc)
    return nc


def _shard_inputs(inputs):
    x = np.ascontiguousarray(np.asarray(inputs["decoder_features"], np.float32))
    y = np.ascontiguousarray(
        np.asarray(inputs["skip_connection_features"], np.float32))
    base = {
        "wq": np.ascontiguousarray(np.asarray(inputs["w_q"], np.float32)),
        "wk": np.ascontiguousarray(np.asarray(inputs["w_k"], np.float32)),
        "wv": np.ascontiguousarray(np.asarray(inputs["w_v"], np.float32)),
        "wo": np.ascontiguousarray(np.asarray(inputs["w_o"], np.float32)),
        "bq": np.asarray(inputs["b_q"], np.float32).reshape(C, 1).copy(),
        "bk": np.asarray(inputs["b_k"], np.float32).reshape(C, 1).copy(),
        "bv": np.asarray(inputs["b_v"], np.float32).reshape(C, 1).copy(),
        "bo": np.asarray(inputs["b_o"], np.float32).reshape(C, 1).copy(),
        "gamma": np.asarray(inputs["gn_gamma"], np.float32).reshape(C, 1).copy(),
        "beta": np.asarray(inputs["gn_beta"], np.float32).reshape(C, 1).copy(),
    }
    in_maps = []
    for ci in range(NCORES):
        sl = slice(HS * ci, HS * (ci + 1))
        im = dict(base)
        im["x"] = np.ascontiguousarray(x[:, :, sl]).reshape(B, C, NVOX)
        im["y"] = np.ascontiguousarray(y[:, :, :, sl]).reshape(B, COND, C, NVOX)
        in_maps.append(im)
    return in_maps


class _Runner:
    """Persistent PJRT runner: trace/compile once, execute many times.

    Mirrors concourse.bass2jax.run_bass_via_pjrt's multi-core branch but
    keeps the jitted shard_map callable alive so repeat calls skip
    re-tracing and NEFF recompilation.
    """

    def __init__(self, nc, donate=True):
        import jax
        from jax.sharding import Mesh, PartitionSpec
        from jax.experimental.shard_map import shard_map
        from concourse import bass2jax, mybir

        bass2jax.install_neuronx_cc_hook()
        assert nc.dbg_addr is None
        partition_name = (nc.partition_id_tensor.name
                          if nc.partition_id_tensor else None)
        in_names, out_names, out_avals, zero_outs = [], [], [], []
        for alloc in nc.m.functions[0].allocations:
            if not isinstance(alloc, mybir.MemoryLocationSet):
                continue
            name = alloc.memorylocations[0].name
            if alloc.kind == "ExternalInput":
                if name != partition_name:
                    in_names.append(name)
            elif alloc.kind == "ExternalOutput":
                out_names.append(name)
                shape = tuple(alloc.tensor_shape)
                dtype = mybir.dt.np(alloc.dtype)
                out_avals.append(jax.core.ShapedArray(shape, dtype))
                zero_outs.append(np.zeros(shape, dtype))
        n_params = len(in_names)
        n_outs = len(out_avals)
        in_names.extend(out_names)
        if partition_name is not None:
            in_names.append(partition_name)
        donate_idx = tuple(range(n_params, n_params + n_outs)) if donate else ()

        def _body(*args):
            operands = list(args)
            if partition_name is not None:
                operands.append(bass2jax.partition_id_tensor())
            outs = bass2jax._bass_exec_p.bind(
                *operands,
                out_avals=tuple(out_avals),
                in_names=tuple(in_names),
                out_names=tuple(out_names),
                lowering_input_output_aliases=(),
                sim_require_finite=True,
                sim_require_nnan=True,
                nc=nc,
            )
            return tuple(outs)

        devices = jax.devices()[:NCORES]
        mesh = Mesh(np.asarray(devices), ("core",))
        in_specs = (PartitionSpec("core"),) * (n_params + n_outs)
        out_specs = (PartitionSpec("core"),) * n_outs
        self._fn = jax.jit(
            shard_map(_body, mesh=mesh, in_specs=in_specs,
                      out_specs=out_specs, check_rep=False),
            donate_argnums=donate_idx, keep_unused=True)
        self._in_names = in_names[:n_params]
        self._out_names = out_names
        self._out_avals = out_avals
        self._zero_outs = zero_outs
        self._jax = jax

    def __call__(self, in_maps):
        concat_in = [
            np.concatenate([np.asarray(m[name]) for m in in_maps], axis=0)
            for name in self._in_names
        ]
        concat_zeros = [
            np.zeros((NCORES * z.shape[0], *z.shape[1:]), z.dtype)
            for z in self._zero_outs
        ]
        out_arrs = self._fn(*concat_in, *concat_zeros)
        out_arrs = self._jax.block_until_ready(out_arrs)
        return [
            {
                name: np.asarray(out_arrs[i]).reshape(
                    NCORES, *self._out_avals[i].shape)[c]
                for i, name in enumerate(self._out_names)
            }
            for c in range(NCORES)
        ]


class _Results:
    def __init__(self, results):
        self.results = results


def _get_runner(n_reps=1, donate=True):
    key = (n_reps, donate)
    if key not in _CACHE:
        _CACHE[key] = _Runner(_build(n_reps), donate=donate)
    return _CACHE[key]


def _run(in_maps, n_reps=1):
    return _Results(_get_runner(n_reps)(in_maps))


def kernel(**inputs) -> np.ndarray:
    res = _run(_shard_inputs(inputs))
    out = np.empty((B, C, H, W, D), np.float32)
    for ci in range(NCORES):
        sl = slice(HS * ci, HS * (ci + 1))
        out[:, :, sl] = res.results[ci]["out"].reshape(B, C, HS, W, D)
    return out


# revision 19
# speedup vs baseline: 1.0190x; 1.0190x over previous
"""Trainium2 Bass kernel for nn_DecoderCrossAttention.

Reference computation (per voxel v, batch b):
    q = Wq x_v + bq                        (x = decoder_features, [C])
    k_j = Wk y_jv + bk, v_j = Wv y_jv + bv (y = skip features, COND=4 frames)
    s_j[h] = <q_h, k_jh> / sqrt(DH)        (NH=8 heads of DH=16)
    attn = softmax_j(s)                    (over the 4 conditioning frames)
    o = Wo (sum_j attn_j * v_j) + bo + x_v
    out = GroupNorm8(o) * gamma + beta     (stats over (C/G, H, W, D) per batch)

Strategy (8 NeuronCores, data-parallel over H; each core: 2*4*32*32 voxels):
  * Feature-major layout [C=128 partitions, voxels free], 512-voxel tiles.
  * All projections are PE matmuls (f32r full-rate at N=512; Wo in bf16).
  * Per-head score reduction and the softmax head->channel broadcast are PE
    matmuls against 0/1 masks built in-kernel with iota+compare.
  * Scores of a 4-tile chunk are packed into ONE [128,512] PSUM bank using
    32-partition strips (tile_position), so exp / Z / 1/Z run once per chunk
    at full 128-partition width:
        esb = exp(S/4); Z = strip-mask matmul (replicated per cond block);
        1/Z = exp(-ln Z) on the scalar engine (vector reciprocal is ~8
        cycles/elem; the two table activations are ~4x cheaper).
  * attn*V products feed 4 accumulating bf16 output-projection matmuls.
  * Residual + bias + per-channel GN sums fused in one scalar_tensor_tensor;
    sum-of-squares via scalar-engine Square with accumulate.
  * GroupNorm is global: per-channel sum/sumsq AllReduce (1KB) per batch,
    batch 0's collective+rescale hidden under batch 1 compute. The rescale
    is in-place on the accumulator, alternating scalar/vector engines.

The walrus build here accepts only ONE sync wait per instruction; Tile
attaches many.  split_waits() hoists extras onto standalone EventSemaphore
instructions post-scheduling.
"""

import sys

if "/opt/trn_rl_repo" not in sys.path:
    sys.path.insert(0, "/opt/trn_rl_repo")

import numpy as np

B, COND, C, H, W, D = 2, 4, 128, 32, 32, 32
NH, DH, G = 8, 16, 8
EPS = 1e-5
NCORES = 8
HS = H // NCORES          # 4 H-planes per core
NVOX = HS * W * D         # 4096 voxels per batch per core
NT = 512                  # voxels per tile
NTILES = NVOX // NT       # 8 tiles per batch
NK = B * NTILES           # 16 tiles per rep
N_GROUP = (C // G) * H * W * D   # elements per (batch, group) for GN stats

_CACHE = {}


def _split_waits(nc):
    """Hoist extra sync waits onto standalone EventSemaphore instructions."""
    from concourse import mybir
    import bass_rust

    n_split = 0
    for func in nc.m.functions:
        for blk in func.blocks:
            new_list = []
            changed = False
            for inst in blk.instructions:
                si = inst.sync_info
                waits = list(si.on_wait) if si is not None else []
                if len(waits) > 1:
                    changed = True
                    for w in waits[:-1]:
                        ev = mybir.InstEventSemaphore(
                            name=f"wsplit-{nc.next_id()}", ins=[], outs=[]
                        )
                        ev.engine = inst.engine
                        ev.sync_info = bass_rust.SyncInfo(on_wait=[w], on_update=[])
                        new_list.append(ev)
                        n_split += 1
                    inst.sync_info = bass_rust.SyncInfo(
                        on_wait=[waits[-1]], on_update=list(si.on_update)
                    )
                new_list.append(inst)
            if changed:
                blk.instructions = new_list
    return n_split


def _build(n_reps=1):
    import concourse.bass as bass
    import concourse.tile as tile
    from concourse import mybir
    from contextlib import ExitStack

    dt = mybir.dt
    f32 = dt.float32
    f32r = dt.float32r
    bf16 = dt.bfloat16
    fp8 = dt.float8e4
    i32 = dt.int32
    Alu = mybir.AluOpType
    Act = mybir.ActivationFunctionType
    ts = bass.ts

    nc = bass.Bass("TRN2", target_bir_lowering=False, debug=False,
                   num_devices=NCORES)
    x_io = nc.dram_tensor("x", [B, C, NVOX], f32r, kind="ExternalInput").ap()
    y_io = nc.dram_tensor("y", [B, COND, C, NVOX], f32r, kind="ExternalInput").ap()
    w_io = {}
    for name in ("wq", "wk", "wv", "wo"):
        w_io[name] = nc.dram_tensor(name, [C, C], f32r, kind="ExternalInput").ap()
    v_io = {}
    for name in ("bq", "bk", "bv", "bo", "gamma", "beta"):
        v_io[name] = nc.dram_tensor(name, [C, 1], f32, kind="ExternalInput").ap()
    out_io = nc.dram_tensor("out", [B, C, NVOX], f32, kind="ExternalOutput").ap()

    def mm(out, lhsT, rhs, start=True, stop=True):
        nc.tensor.matmul(out, lhsT=lhsT, rhs=rhs, start=start, stop=stop)

    with tile.TileContext(nc) as tc, ExitStack() as ctx:
        # ---------------- constants / weights / masks -------------------
        const = ctx.enter_context(tc.tile_pool(name="const", bufs=1))
        dram = ctx.enter_context(tc.tile_pool(name="dram", bufs=1, space="DRAM"))

        vecs = {}
        for name, io in v_io.items():
            t = const.tile([C, 1], f32, tag=f"vec_{name}")
            nc.sync.dma_start(t[:], io[:])
            vecs[name] = t

        with tc.tile_pool(name="setup", bufs=1) as setup:
            def icast(dst_ap, src_ap):
                nc.vector.tensor_copy(dst_ap, src_ap)

            # partition-index and free-index helpers
            p128 = setup.tile([C, C], i32, tag="p128")
            nc.gpsimd.iota(p128[:], pattern=[[0, C]], base=0, channel_multiplier=1)
            f128 = setup.tile([C, C], i32, tag="f128")
            nc.gpsimd.iota(f128[:], pattern=[[1, C]], base=0, channel_multiplier=0)
            hc128 = setup.tile([C, C], i32, tag="hc128")
            nc.vector.tensor_scalar(hc128[:], p128[:], 4, None,
                                    Alu.arith_shift_right)
            tmpi = setup.tile([C, C], i32, tag="tmpi")

            # identity [128,128] (for PE transpose of the weights)
            ident = const.tile([C, C], f32r, tag="ident")
            nc.vector.tensor_tensor(tmpi[:], f128[:], p128[:], Alu.is_equal)
            icast(ident[:], tmpi[:])

            # All matmul outputs must start at partition 0, so the per-chunk
            # score / broadcast masks are full-height [128,128] slices, one
            # per (tile-strip u, cond j), packed along free as [128, 16*128].
            NM = 4 * COND * C
            pfull = setup.tile([C, NM], i32, tag="pfull")
            nc.gpsimd.iota(pfull[:], pattern=[[0, NM]], base=0,
                           channel_multiplier=1)
            p16f = setup.tile([C, NM], i32, tag="p16f")
            nc.vector.tensor_scalar(p16f[:], pfull[:], 4, None,
                                    Alu.arith_shift_right)
            # base(u,j) = 32u + 8j replicated over the 128-col block
            ujb = setup.tile([C, NM], i32, tag="ujb")
            nc.gpsimd.iota(ujb[:].rearrange("p (u j c) -> p u j c", u=4, j=4),
                           pattern=[[32, 4], [8, 4], [0, C]], base=0,
                           channel_multiplier=0)
            # r = col index within each 128 block
            rcol = setup.tile([C, NM], i32, tag="rcol")
            nc.gpsimd.iota(rcol[:].rearrange("p (b c) -> p b c", b=16),
                           pattern=[[0, 16], [1, C]], base=0,
                           channel_multiplier=0)
            # mask32dr[p, (u,j,r)] = 1 iff r == 32u + 8j + p//16 : fp8 score
            # lhsT, consumed as DoubleRow cond-pairs [(u,pair), k=2, 128]
            tgt = setup.tile([C, NM], i32, tag="tgt")
            nc.vector.tensor_tensor(tgt[:], ujb[:], p16f[:], Alu.add)
            ebig = setup.tile([C, NM], i32, tag="ebig")
            nc.vector.tensor_tensor(ebig[:], rcol[:], tgt[:], Alu.is_equal)
            efl = setup.tile([C, NM], f32, tag="efl")
            nc.vector.tensor_copy(efl[:], ebig[:])
            mask32dr = const.tile([C, NM], fp8, tag="mask32dr")
            nc.scalar.copy(mask32dr[:], efl[:])

            # maskbf[p, (u,j,c)] = 1 iff p == 32u + 8j + c//16  (bcast lhsT)
            r16 = setup.tile([C, NM], i32, tag="r16")
            nc.vector.tensor_scalar(r16[:], rcol[:], 4, None,
                                    Alu.arith_shift_right)
            nc.vector.tensor_tensor(tgt[:], ujb[:], r16[:], Alu.add)
            nc.vector.tensor_tensor(ebig[:], pfull[:], tgt[:], Alu.is_equal)
            maskbf = const.tile([C, NM], bf16, tag="maskbf")
            icast(maskbf[:], ebig[:])

            # zquad [128,128]: 1 iff p//32==r//32 and p%8==r%8 (chunk Z matmul)
            rr = setup.tile([C, C], i32, tag="rr")
            nc.gpsimd.iota(rr[:], pattern=[[1, C]], base=0, channel_multiplier=0)
            za = setup.tile([C, C], i32, tag="za")
            nc.vector.tensor_scalar(za[:], rr[:], 5, None, Alu.arith_shift_right)
            zp = setup.tile([C, C], i32, tag="zp")
            nc.vector.tensor_scalar(zp[:], p128[:], 5, None, Alu.arith_shift_right)
            zeq1 = setup.tile([C, C], i32, tag="zeq1")
            nc.vector.tensor_tensor(zeq1[:], za[:], zp[:], Alu.is_equal)
            zm1 = setup.tile([C, C], i32, tag="zm1")
            nc.vector.tensor_scalar(zm1[:], rr[:], 3, 3,
                                    Alu.arith_shift_right, Alu.arith_shift_left)
            nc.vector.tensor_tensor(zm1[:], rr[:], zm1[:], Alu.subtract)
            zm2 = setup.tile([C, C], i32, tag="zm2")
            nc.vector.tensor_scalar(zm2[:], p128[:], 3, 3,
                                    Alu.arith_shift_right, Alu.arith_shift_left)
            nc.vector.tensor_tensor(zm2[:], p128[:], zm2[:], Alu.subtract)
            zeq2 = setup.tile([C, C], i32, tag="zeq2")
            nc.vector.tensor_tensor(zeq2[:], zm1[:], zm2[:], Alu.is_equal)
            nc.vector.tensor_tensor(zeq1[:], zeq1[:], zeq2[:], Alu.mult)
            zquad = const.tile([C, C], f32r, tag="zquad")
            icast(zquad[:], zeq1[:])

            # gmask [128, 8]: 1 iff c//16 == g   (GN group reduction)
            g8 = setup.tile([C, 8], i32, tag="g8")
            nc.gpsimd.iota(g8[:], pattern=[[1, 8]], base=0, channel_multiplier=0)
            e8 = setup.tile([C, 8], i32, tag="e8")
            nc.vector.tensor_tensor(e8[:], g8[:], hc128[:, 0:8], Alu.is_equal)
            gmask = const.tile([C, 8], f32, tag="gmask")
            icast(gmask[:], e8[:])

            # gm2 [8, 128]: 1 iff p == c//16    (GN group -> channel bcast)
            p8 = setup.tile([8, C], i32, tag="p8")
            nc.gpsimd.iota(p8[:], pattern=[[0, C]], base=0, channel_multiplier=1)
            fc8 = setup.tile([8, C], i32, tag="fc8")
            nc.gpsimd.iota(fc8[:], pattern=[[1, C]], base=0, channel_multiplier=0)
            nc.vector.tensor_scalar(fc8[:], fc8[:], 4, None, Alu.arith_shift_right)
            e82 = setup.tile([8, C], i32, tag="e82")
            nc.vector.tensor_tensor(e82[:], p8[:], fc8[:], Alu.is_equal)
            gm2 = const.tile([8, C], f32, tag="gm2")
            icast(gm2[:], e82[:])

            # --- load + transpose the four projection weights
            wT = {}
            with tc.tile_pool(name="psum_setup", bufs=1, space="PSUM") as psum_su:
                raws = {}
                for name in ("wq", "wk", "wv", "wo"):
                    raw = setup.tile([C, C], f32r, tag=f"raw_{name}")
                    nc.sync.dma_start(raw[:], w_io[name][:])
                    raws[name] = raw
                for name in ("wq", "wk", "wv", "wo"):
                    pst = psum_su.tile([C, C], f32r, tag=f"pst_{name}")
                    nc.tensor.transpose(pst[:], raws[name][:], ident[:])
                    t = const.tile([C, C], bf16 if name == "wo" else f32r,
                                   tag=f"wT_{name}")
                    nc.scalar.copy(t[:], pst[:])
                    wT[name] = t

            # warm the activation tables used by the steady state
            warm = setup.tile([C, 2], f32, tag="warm")
            nc.vector.memset(warm[:, 0:1], 1.0)
            nc.scalar.activation(warm[:, 1:2], warm[:, 0:1], Act.Exp)
            nc.scalar.activation(warm[:, 1:2], warm[:, 0:1], Act.Ln)
            nc.scalar.activation(warm[:, 1:2], warm[:, 0:1], Act.Square)
            nc.scalar.activation(warm[:, 1:2], warm[:, 0:1], Act.Sqrt)
            nc.scalar.activation(warm[:, 1:2], warm[:, 0:1], Act.Identity)

        # ---------------- main pipeline ---------------------------------
        p = dict(
            xres=ctx.enter_context(tc.tile_pool(name="xres", bufs=2)),
            ypool=ctx.enter_context(tc.tile_pool(name="ypool", bufs=2)),
            qpool=ctx.enter_context(tc.tile_pool(name="qpool", bufs=2)),
            qkpool=ctx.enter_context(tc.tile_pool(name="qkpool", bufs=2)),
            vpool=ctx.enter_context(tc.tile_pool(name="vpool", bufs=5)),
            wpool=ctx.enter_context(tc.tile_pool(name="wpool", bufs=2)),
            soft=ctx.enter_context(tc.tile_pool(name="soft", bufs=2)),
            opool=ctx.enter_context(tc.tile_pool(name="opool", bufs=1)),
            stats=ctx.enter_context(tc.tile_pool(name="stats", bufs=1)),
            ps_k=ctx.enter_context(tc.tile_pool(name="ps_k", bufs=2, space="PSUM")),
            ps_q=ctx.enter_context(tc.tile_pool(name="ps_q", bufs=1, space="PSUM")),
            ps_v=ctx.enter_context(tc.tile_pool(name="ps_v", bufs=1, space="PSUM")),
            ps_sz=ctx.enter_context(tc.tile_pool(name="ps_sz", bufs=1, space="PSUM")),
            ps_bb=ctx.enter_context(tc.tile_pool(name="ps_bb", bufs=2, space="PSUM")),
            ps_o=ctx.enter_context(tc.tile_pool(name="ps_o", bufs=1, space="PSUM")),
        )

        # chunk layout: 4 tiles per chunk; chunk-local index u selects the
        # 32-partition strip of the packed score / Z tiles
        def chunk_of(t):
            return t // 4, t % 4

        NCH = NTILES // 4          # chunks per batch

        for rep in range(n_reps):
            out_acc = p["opool"].tile([C, B * NVOX], f32, tag="out_acc")
            sums = p["stats"].tile([C, B * NTILES], f32, tag="sums")
            ssqs = p["stats"].tile([C, B * NTILES], f32, tag="ssqs")
            dump = p["stats"].tile([C, NT], f32, tag="dump")

            tiles = [(b, t) for b in range(B) for t in range(NTILES)]
            xres_b = {}
            ych_state = {}
            fstate = {}
            chunk_ps = {}
            sstate = {}
            cc_state = {}

            def front(k):
                b, t = tiles[k]
                if t == 0:
                    xr = p["xres"].tile([C, NVOX], f32r, tag="xres")
                    nc.sync.dma_start(xr[:, 0: NVOX // 2],
                                      x_io[b][:, 0: NVOX // 2])
                    nc.sync.dma_start(xr[:, NVOX // 2: NVOX],
                                      x_io[b][:, NVOX // 2: NVOX])
                    xres_b[b] = xr
                if t % 2 == 0:
                    ych = p["ypool"].tile([C, COND * 2 * NT], f32r, tag="ych")
                    ysrc = y_io[b].rearrange("j c v -> c j v")
                    nc.sync.dma_start(
                        ych[:].rearrange("p (j v) -> p j v", j=COND),
                        ysrc[:, :, ts(t // 2, 2 * NT)])
                    ych_state[b] = ych
                ych = ych_state[b]
                yj = lambda j: ych[:, j * 2 * NT + (t % 2) * NT:
                               j * 2 * NT + (t % 2 + 1) * NT]
                xt = xres_b[b][:, ts(t, NT)]
                ci, u = chunk_of(t)
                cid = b * NCH + ci

                psQ = p["ps_q"].tile([C, NT], f32, tag="q")
                mm(psQ[:], wT["wq"][:], xt)
                qsb = p["qpool"].tile([C, NT], f32, tag="qsb")
                nc.scalar.activation(qsb[:], psQ[:], Act.Identity,
                                     bias=vecs["bq"][:])

                qk = p["qkpool"].tile([C, COND * NT], fp8, tag="qk")
                for j in range(COND):
                    psK = p["ps_k"].tile([C, NT], f32, tag="k")
                    mm(psK[:], wT["wk"][:], yj(j))
                    nc.vector.scalar_tensor_tensor(
                        qk[:, ts(j, NT)], psK[:], vecs["bk"][:], qsb[:],
                        Alu.add, Alu.mult)

                if u == 0:
                    psS4c = p["ps_sz"].tile([C, NT], f32, tag="sz", name="psS4")
                    chunk_ps[cid] = psS4c
                psS4 = chunk_ps[cid]
                for pr in range(2):
                    nc.tensor.matmul(
                        psS4[:],
                        lhsT=mask32dr[:, ts(u * 2 + pr, 2 * C)]
                        .rearrange("p (k m) -> p k m", k=2),
                        rhs=qk[:, ts(pr, 2 * NT)]
                        .rearrange("p (k n) -> p k n", k=2),
                        start=(u == 0 and pr == 0),
                        stop=(u == 3 and pr == 1),
                        perf_mode=mybir.MatmulPerfMode.DoubleRow,
                        skip_group_check=True)

                vb = p["vpool"].tile([C, COND * NT], bf16, tag="vb")
                for j in range(COND):
                    psV = p["ps_v"].tile([C, NT], f32, tag="v")
                    mm(psV[:], wT["wv"][:], yj(j))
                    nc.scalar.activation(vb[:, ts(j, NT)], psV[:],
                                         Act.Identity, bias=vecs["bv"][:])
                fstate[k] = (xt, vb)

            def softchunk(cid):
                psS4 = chunk_ps.pop(cid)
                esb = p["soft"].tile([C, NT], f32r, tag="esb")
                nc.scalar.activation(esb[:], psS4[:], Act.Exp, scale=0.25)
                psZ = p["ps_sz"].tile([C, NT], f32, tag="sz")
                mm(psZ[:], zquad[:], esb[:])
                lnz = p["soft"].tile([C, NT], f32, tag="lnz")
                nc.scalar.activation(lnz[:], psZ[:], Act.Ln)
                rsb = p["soft"].tile([C, NT], f32, tag="rsb")
                nc.scalar.activation(rsb[:], lnz[:], Act.Exp, scale=-1.0)
                etsb = p["soft"].tile([C, NT], bf16, tag="etsb")
                nc.vector.tensor_tensor(etsb[:], esb[:].bitcast(f32), rsb[:],
                                        Alu.mult)
                sstate[cid] = etsb

            def back(k):
                b, t = tiles[k]
                ci, u = chunk_of(t)
                xt, vb = fstate.pop(k)
                etsb = sstate[b * NCH + ci]
                wb = p["wpool"].tile([C, COND * NT], bf16, tag="wb")
                for j in range(COND):
                    psBB = p["ps_bb"].tile([C, NT], f32, tag="bb")
                    mm(psBB[:], maskbf[:, ts(u * COND + j, C)], etsb[:])
                    nc.vector.tensor_tensor(wb[:, ts(j, NT)], psBB[:],
                                            vb[:, ts(j, NT)], Alu.mult)
                psO = p["ps_o"].tile([C, NT], f32, tag="o")
                for j in range(COND):
                    mm(psO[:], wT["wo"][:], wb[:, ts(j, NT)],
                       start=(j == 0), stop=(j == COND - 1))
                col = b * NTILES + t
                outt = out_acc[:, col * NT: (col + 1) * NT]
                nc.vector.scalar_tensor_tensor(
                    outt, psO[:], vecs["bo"][:], xt.bitcast(f32),
                    Alu.add, Alu.add,
                    accum_out=sums[:, col: col + 1])
                nc.scalar.activation(
                    dump[:], outt, Act.Square,
                    accum_out=ssqs[:, col: col + 1])

            def gn_pre(b):
                """Reduce per-channel stats and launch the AllReduce."""
                ccsb = p["stats"].tile([C, 2], f32, tag=f"ccsb{b}")
                nc.vector.reduce_sum(ccsb[:, 0:1],
                                     sums[:, b * NTILES:(b + 1) * NTILES],
                                     axis=mybir.AxisListType.X)
                nc.vector.reduce_sum(ccsb[:, 1:2],
                                     ssqs[:, b * NTILES:(b + 1) * NTILES],
                                     axis=mybir.AxisListType.X)
                cc_in = dram.tile([C, 2], f32, tag=f"cc_in{b}")
                cc_out = dram.tile([C, 2], f32, tag=f"cc_out{b}")
                nc.sync.dma_start(cc_in[:], ccsb[:])
                nc.gpsimd.collective_compute(
                    "AllReduce", Alu.add,
                    replica_groups=[list(range(NCORES))],
                    ins=[cc_in.opt()], outs=[cc_out.opt()])
                cc_state[b] = cc_out

            def gn_post(b):
                """Stats -> per-channel affine -> rescale out_acc -> store."""
                cc_out = cc_state.pop(b)
                gsb = p["stats"].tile([C, 2], f32, tag=f"gsb{b}")
                nc.sync.dma_start(gsb[:], cc_out[:])
                psG = p["ps_q"].tile([8, 2], f32, tag="q")
                nc.tensor.matmul(psG[:], lhsT=gmask[:], rhs=gsb[:],
                                 start=True, stop=True)
                msb = p["stats"].tile([8, 2], f32, tag=f"msb{b}")
                nc.vector.tensor_scalar(msb[:], psG[:], 1.0 / N_GROUP, None,
                                        Alu.mult)
                vtmp = p["stats"].tile([8, 2], f32, tag=f"vtmp{b}")
                eps_t = p["stats"].tile([8, 1], f32, tag=f"eps{b}")
                nc.vector.memset(eps_t[:], EPS)
                nc.vector.tensor_tensor(vtmp[:, 0:1], msb[:, 0:1],
                                        msb[:, 0:1], Alu.mult)
                nc.vector.tensor_tensor(vtmp[:, 1:2], msb[:, 1:2],
                                        vtmp[:, 0:1], Alu.subtract)
                nc.scalar.activation(vtmp[:, 0:1], vtmp[:, 1:2], Act.Sqrt,
                                     bias=eps_t[:])
                pstat = p["stats"].tile([8, 2], f32, tag=f"pstat{b}")
                nc.vector.tensor_copy(pstat[:, 0:1], msb[:, 0:1])
                nc.vector.reciprocal(pstat[:, 1:2], vtmp[:, 0:1])
                psP = p["ps_q"].tile([C, 2], f32, tag="q")
                nc.tensor.matmul(psP[:], lhsT=gm2[:], rhs=pstat[:],
                                 start=True, stop=True)
                scale_b = p["stats"].tile([C, 1], f32, tag=f"scale{b}")
                nc.vector.tensor_tensor(scale_b[:], psP[:, 1:2],
                                        vecs["gamma"][:], Alu.mult)
                negb_b = p["stats"].tile([C, 1], f32, tag=f"negb{b}")
                nc.vector.scalar_tensor_tensor(
                    negb_b[:], psP[:, 0:1], scale_b[:],
                    vecs["beta"][:], Alu.mult, Alu.subtract)
                nb2 = p["stats"].tile([C, 1], f32, tag=f"nb2{b}")
                nc.vector.tensor_scalar(nb2[:], negb_b[:], -1.0, None,
                                        Alu.mult)
                for t in range(NTILES):
                    seg = out_acc[:, (b * NTILES + t) * NT:
                                  (b * NTILES + t + 1) * NT]
                    if t % 2 == 0:
                        nc.scalar.activation(seg, seg, Act.Identity,
                                             bias=nb2[:], scale=scale_b[:])
                    else:
                        nc.vector.tensor_scalar(
                            seg, seg, scale_b[:], negb_b[:],
                            Alu.mult, Alu.subtract)
                    if t % 4 == 3:
                        half = 4 * NT
                        hi = (t // 4)
                        nc.sync.dma_start(
                            out_io[b][:, hi * half: (hi + 1) * half],
                            out_acc[:, b * NVOX + hi * half:
                                    b * NVOX + (hi + 1) * half])

            for k in range(NK + 4):
                if k < NK:
                    front(k)
                if k >= 4:
                    back(k - 4)
                    bdone, tdone = tiles[k - 4]
                    if tdone == NTILES - 1:
                        gn_pre(bdone)
                if k < NK:
                    bk_, tk_ = tiles[k]
                    ci_, u_ = chunk_of(tk_)
                    if u_ == 3:
                        softchunk(bk_ * NCH + ci_)
                # hide batch-0 AllReduce under batch-1 compute
                if k == NK - 1 and 0 in cc_state:
                    gn_post(0)
            gn_post(B - 1)

    _split_waits(nc)
    return nc


def _shard_inputs(inputs):
    x = np.ascontiguousarray(np.asarray(inputs["decoder_features"], np.float32))
    y = np.ascontiguousarray(
        np.asarray(inputs["skip_connection_features"], np.float32))
    base = {
        "wq": np.ascontiguousarray(np.asarray(inputs["w_q"], np.float32)),
        "wk": np.ascontiguousarray(np.asarray(inputs["w_k"], np.float32)),
        "wv": np.ascontiguousarray(np.asarray(inputs["w_v"], np.float32)),
        "wo": np.ascontiguousarray(np.asarray(inputs["w_o"], np.float32)),
        "bq": np.asarray(inputs["b_q"], np.float32).reshape(C, 1).copy(),
        "bk": np.asarray(inputs["b_k"], np.float32).reshape(C, 1).copy(),
        "bv": np.asarray(inputs["b_v"], np.float32).reshape(C, 1).copy(),
        "bo": np.asarray(inputs["b_o"], np.float32).reshape(C, 1).copy(),
        "gamma": np.asarray(inputs["gn_gamma"], np.float32).reshape(C, 1).copy(),
        "beta": np.asarray(inputs["gn_beta"], np.float32).reshape(C, 1).copy(),
    }
    in_maps = []
    for ci in range(NCORES):
        sl = slice(HS * ci, HS * (ci + 1))
        im = dict(base)
        im["x"] = np.ascontiguousarray(x[:, :, sl]).reshape(B, C, NVOX)
        im["y"] = np.ascontiguousarray(y[:, :, :, sl]).reshape(B, COND, C, NVOX)
        in_maps.append(im)
    return in_maps


class _Runner:
    """Persistent PJRT runner: trace/compile once, execute many times.

    Mirrors concourse.bass2jax.run_bass_via_pjrt's multi-core branch but
    keeps the jitted shard_map callable alive so repeat calls skip
    re-tracing and NEFF recompilation.
    """

    def __init__(self, nc, donate=True):
        import jax
        from jax.sharding import Mesh, PartitionSpec
        from jax.experimental.shard_map import shard_map
        from concourse import bass2jax, mybir

        bass2jax.install_neuronx_cc_hook()
        assert nc.dbg_addr is None
        partition_name = (nc.partition_id_tensor.name
                          if nc.partition_id_tensor else None)
        in_names, out_names, out_avals, zero_outs = [], [], [], []
        for alloc in nc.m.functions[0].allocations:
            if not isinstance(alloc, mybir.MemoryLocationSet):
                continue
            name = alloc.memorylocations[0].name
            if alloc.kind == "ExternalInput":
                if name != partition_name:
                    in_names.append(name)
            elif alloc.kind == "ExternalOutput":
                out_names.append(name)
                shape = tuple(alloc.tensor_shape)
                dtype = mybir.dt.np(alloc.dtype)
                out_avals.append(jax.core.ShapedArray(shape, dtype))
                zero_outs.append(np.zeros(shape, dtype))
        n_params = len(in_names)
        n_outs = len(out_avals)
        in_names.extend(out_names)
        if partition_name is not None:
            in_names.append(partition_name)
        donate_idx = tuple(range(n_params, n_params + n_outs)) if donate else ()

        def _body(*args):
            operands = list(args)
            if partition_name is not None:
                operands.append(bass2jax.partition_id_tensor())
            outs = bass2jax._bass_exec_p.bind(
                *operands,
                out_avals=tuple(out_avals),
                in_names=tuple(in_names),
                out_names=tuple(out_names),
                lowering_input_output_aliases=(),
                sim_require_finite=True,
                sim_require_nnan=True,
                nc=nc,
            )
            return tuple(outs)

        devices = jax.devices()[:NCORES]
        mesh = Mesh(np.asarray(devices), ("core",))
        in_specs = (PartitionSpec("core"),) * (n_params + n_outs)
        out_specs = (PartitionSpec("core"),) * n_outs
        self._fn = jax.jit(
            shard_map(_body, mesh=mesh, in_specs=in_specs,
                      out_specs=out_specs, check_rep=False),
            donate_argnums=donate_idx, keep_unused=True)
        self._in_names = in_names[:n_params]
        self._out_names = out_names
        self._out_avals = out_avals
        self._zero_outs = zero_outs
        self._jax = jax

    def __call__(self, in_maps):
        concat_in = [
            np.concatenate([np.asarray(m[name]) for m in in_maps], axis=0)
            for name in self._in_names
        ]
        concat_zeros = [
            np.zeros((NCORES * z.shape[0], *z.shape[1:]), z.dtype)
            for z in self._zero_outs
        ]
        out_arrs = self._fn(*concat_in, *concat_zeros)
        out_arrs = self._jax.block_until_ready(out_arrs)
        return [
            {
                name: np.asarray(out_arrs[i]).reshape(
                    NCORES, *self._out_avals[i].shape)[c]
                for i, name in enumerate(self._out_names)
            }
            for c in range(NCORES)
        ]


class _Results:
    def __init__(self, results):
        self.results = results


def _get_runner(n_reps=1, donate=True):
    key = (n_reps, donate)
    if key not in _CACHE:
        _CACHE[key] = _Runner(_build(n_reps), donate=donate)
    return _CACHE[key]


def _run(in_maps, n_reps=1):
    return _Results(_get_runner(n_reps)(in_maps))


def kernel(**inputs) -> np.ndarray:
    res = _run(_shard_inputs(inputs))
    out = np.empty((B, C, H, W, D), np.float32)
    for ci in range(NCORES):
        sl = slice(HS * ci, HS * (ci + 1))
        out[:, :, sl] = res.results[ci]["out"].reshape(B, C, HS, W, D)
    return out
